# revision 9
# baseline (speedup 1.0000x reference)
"""AlignNet (dense CNN + DCNv2) Trainium2 Bass kernel, 8 NeuronCores.

Sharding: data-parallel over (batch, H-half): core c=(b,h) computes output
rows [0:96)/[96:192) of batch b with a 16-row replicated halo (no
inter-core communication).

Transfer-optimized I/O (the axon tunnel is the bottleneck):
  - frame activations shipped as per-(batch,channel)-scaled int8, dequantized
    on device by ActE with a per-partition AP scale -> bf16 canvases
  - all weights packed into one bf16 blob, unpacked by strided DMA views
  - output returned as bf16; donated output buffer lives on device between
    calls so no zero-upload is needed
  - the jitted shard_map executable is cached across kernel() calls

Per-core pipeline (bf16 compute, fp32 PSUM):
  - activations in padded DRAM canvases [C, 118, 324] bf16 (image origin
    (2,2); borders zero = conv/sampling zero-pad)
  - 3x3 convs: 9 (or 5 tap-paired) accumulated matmuls on shifted flat views
  - DCNv2: offsets clipped to (-1,1) -> exact 3x3 hat window; per-(g,k)
    window weights on 72 partitions, replicated to channel layout by
    SBUF->SBUF DMAs, DVE products, 9-cell reduction + channel einsum
    absorbed into TensorE matmuls.
"""
import numpy as np
import ml_dtypes

NF, DG, KK = 64, 8, 9
B, H, W = 4, 192, 320
RR = 112                  # compute rows per core (96 + 16 halo)
CH, CW = RR + 6, W + 4    # canvas 118 x 324, image origin (2,2)
CWH = CH * CW
GUARD = 8
SLACK = 336
BF = ml_dtypes.bfloat16

# weight blob layout: (name, shape) in fixed order
WSPEC = [
    ("w1", (128, 9, 128)), ("b1", (1, 128)),
    ("w2", (128, 9, 128)), ("b2", (1, 128)),
    ("womA", (128, 5, 72)), ("womB", (128, 5, 72)), ("womC", (128, 5, 72)),
    ("bomA", (1, 72)), ("bomB", (1, 72)), ("bomC", (1, 72)),
    ("wd", (128, 9, 128)), ("bd", (1, 128)),
    ("wf1", (128, 9, 64)), ("bf1", (1, 64)),
    ("wf2", (128, 5, 64)), ("bf2", (1, 64)),
]
WOFF = {}
_o = 0
for _n, _s in WSPEC:
    WOFF[_n] = _o
    _o += int(np.prod(_s))
NW = _o

_cache = {}


def _build():
    import concourse.bass as bass
    import concourse.bacc as bacc
    import concourse.mybir as mybir
    from concourse import tile

    F32 = mybir.dt.float32
    BF16 = mybir.dt.bfloat16
    I8 = mybir.dt.int8
    AF = mybir.ActivationFunctionType
    ALU = mybir.AluOpType

    nc = bacc.Bacc("TRN2", target_bir_lowering=False, debug=False)

    feaq = [nc.declare_dram_parameter(f"feaq{i}", [64, RR, W], I8, isOutput=False)
            for i in range(5)]
    fscale = nc.declare_dram_parameter("fscale", [64, 8], F32, isOutput=False)
    wblob = nc.declare_dram_parameter("wblob", [NW], BF16, isOutput=False)
    out_p = nc.declare_dram_parameter("out", [64, RR, W], BF16, isOutput=True)

    def canvas(name, ch):
        return nc.dram_tensor(name, [ch, CH, CW], BF16)

    cv_in = [canvas(f"cv_fea{i}", 64) for i in range(5)]
    cv_b1 = canvas("cv_b1", 64)
    cv_b2 = canvas("cv_b2", 64)
    cv_b3 = canvas("cv_b3", 64)
    cv_q1 = canvas("cv_q1", 128)
    cv_q2 = canvas("cv_q2", 128)
    cv_dd = canvas("cv_dd", 128)
    cv_g = canvas("cv_g", 64)

    with tile.TileContext(nc) as tc:
        with tc.tile_pool(name="wgt", bufs=1) as wgt:
            # ---- unpack bf16 weights from the blob ----
            wt = {}
            for name, shp in WSPEC:
                p_, a_ = shp[0], shp[1]
                b_ = shp[2] if len(shp) == 3 else None
                t16 = wgt.tile(list(shp), BF16, tag=f'w_{name}', name=f'w_{name}')
                if b_ is None:
                    src = bass.AP(wblob[:].tensor, WOFF[name], [[a_, p_], [1, a_]])
                else:
                    src = bass.AP(wblob[:].tensor, WOFF[name],
                                  [[a_ * b_, p_], [b_, a_], [1, b_]])
                nc.sync.dma_start(t16[:], src)
                wt[name] = t16
            fst = wgt.tile([64, 8], F32, tag="fst")
            nc.sync.dma_start(fst[:], fscale[:])
            ones = wgt.tile([1, CW], BF16)
            nc.gpsimd.memset(ones[:], 1.0)

            # ---- zero canvases + dequantize inputs into canvases ----
            with tc.tile_pool(name="init", bufs=2) as ip:
                zt = ip.tile([128, 8192], BF16, tag="zt")
                nc.gpsimd.memset(zt[:], 0.0)
                for cv, ch in ([(c, 64) for c in cv_in] +
                               [(cv_b1, 64), (cv_b2, 64), (cv_b3, 64), (cv_g, 64),
                                (cv_q1, 128), (cv_q2, 128), (cv_dd, 128)]):
                    flat = cv[:].rearrange("c h w -> c (h w)")
                    for o in range(0, CWH, 8192):
                        n = min(8192, CWH - o)
                        nc.sync.dma_start(flat[0:ch, o:o + n], zt[0:ch, 0:n])
                for i in range(5):
                    for r0 in range(0, RR, 16):
                        ti8 = ip.tile([64, 16 * W], I8, tag="qi")
                        src = bass.AP(feaq[i][:].tensor, r0 * W,
                                      [[RR * W, 64], [1, 16 * W]])
                        nc.sync.dma_start(ti8[:], src)
                        t16 = ip.tile([64, 16 * W], BF16, tag="qc")
                        nc.scalar.mul(t16[:], ti8[:], fst[:, i:i + 1])
                        dst = bass.AP(cv_in[i][:].tensor, (r0 + 2) * CW + 2,
                                      [[CWH, 64], [CW, 16], [1, W]])
                        nc.sync.dma_start(dst, t16[:].rearrange("c (r w) -> c r w", r=16))

            # ============ stage helpers ============
            def conv_stage(src_list, dst, w_name, b_name, mout):
                BAND = 8
                wtile = wt[w_name]
                btile = wt[b_name]
                with (tc.tile_pool(name="cs", bufs=2) as sp,
                      tc.tile_pool(name="cps", bufs=3, space="PSUM") as pp):
                    for b0 in range(0, RR, BAND):
                        rows = BAND + 2
                        pitch = GUARD + rows * CW + SLACK
                        xt = sp.tile([128, pitch], BF16, tag="cx")
                        base = (b0 + 1) * CW
                        if len(src_list) == 1:
                            sf = src_list[0][:].rearrange("c h w -> c (h w)")
                            nc.sync.dma_start(xt[:, GUARD:GUARD + rows * CW],
                                              sf[:, base:base + rows * CW])
                        else:
                            for hh in (0, 1):
                                sf = src_list[hh][:].rearrange("c h w -> c (h w)")
                                nc.sync.dma_start(xt[64 * hh:64 * hh + 64, GUARD:GUARD + rows * CW],
                                                  sf[:, base:base + rows * CW])
                        otile = sp.tile([mout, BAND, CW], BF16, tag="co")
                        for r in range(BAND):
                            acc = pp.tile([mout, CW], F32, tag="cp")
                            for tap in range(9):
                                ky, kx = tap // 3 - 1, tap % 3 - 1
                                off = GUARD + (r + 1 + ky) * CW + kx
                                rhs = bass.AP(xt[:].tensor, off, [[pitch, 128], [1, CW]])
                                nc.tensor.matmul(acc[:], wtile[:, tap, 0:mout], rhs,
                                                 start=(tap == 0), stop=False)
                            nc.tensor.matmul(acc[:], btile[:, 0:mout], ones[:],
                                             start=False, stop=True)
                            nc.scalar.activation(otile[:, r, :], acc[:], AF.Prelu, alpha=0.1)
                        if dst is None:
                            dd = bass.AP(out_p[:].tensor, b0 * W,
                                         [[RR * W, 64], [W, BAND], [1, W]])
                        else:
                            dd = bass.AP(dst[:].tensor, (b0 + 2) * CW + 2,
                                         [[CWH, mout], [CW, BAND], [1, W]])
                        sv = bass.AP(otile[:].tensor, 2,
                                     [[BAND * CW, mout], [CW, BAND], [1, W]])
                        nc.sync.dma_start(dd, sv)

            def pair_conv_stage(src, dst, w_name, b_name, mout):
                BAND = 8
                wtile = wt[w_name]
                btile = wt[b_name]
                sflat = src[:].rearrange("c h w -> c (h w)")
                with (tc.tile_pool(name="pcs", bufs=2) as sp,
                      tc.tile_pool(name="pps", bufs=3, space="PSUM") as pp):
                    for b0 in range(0, RR, BAND):
                        rows = BAND + 2
                        base = (b0 + 1) * CW
                        pitch = GUARD + rows * CW + SLACK
                        t1 = sp.tile([128, pitch], BF16, tag="p1")
                        nc.sync.dma_start(t1[0:64, GUARD:GUARD + rows * CW],
                                          sflat[:, base:base + rows * CW])
                        nc.sync.dma_start(t1[64:128, GUARD:GUARD + rows * CW],
                                          sflat[:, base + 1:base + 1 + rows * CW])
                        t2 = sp.tile([128, pitch], BF16, tag="p2")
                        nc.sync.dma_start(t2[0:64, GUARD:GUARD + rows * CW],
                                          sflat[:, base:base + rows * CW])
                        nc.sync.dma_start(t2[64:128, GUARD:GUARD + rows * CW],
                                          sflat[:, base + CW:base + CW + rows * CW])
                        otile = sp.tile([mout, BAND, CW], BF16, tag="po")
                        for r in range(BAND):
                            acc = pp.tile([mout, CW], F32, tag="pp")
                            first = True
                            for s, ky in enumerate((-1, 0, 1)):
                                off = GUARD + (r + 1 + ky) * CW - 1
                                rhs = bass.AP(t1[:].tensor, off, [[pitch, 128], [1, CW]])
                                nc.tensor.matmul(acc[:], wtile[:, s, 0:mout], rhs,
                                                 start=first, stop=False)
                                first = False
                            off = GUARD + r * CW + 1
                            rhs = bass.AP(t2[:].tensor, off, [[pitch, 128], [1, CW]])
                            nc.tensor.matmul(acc[:], wtile[:, 3, 0:mout], rhs, start=False, stop=False)
                            off = GUARD + (r + 2) * CW + 1
                            rhs = bass.AP(t1[:].tensor, off, [[pitch, 128], [1, CW]])
                            nc.tensor.matmul(acc[:], wtile[:, 4, 0:mout], rhs, start=False, stop=False)
                            nc.tensor.matmul(acc[:], btile[:, 0:mout], ones[:], start=False, stop=True)
                            nc.scalar.activation(otile[:, r, :], acc[:], AF.Prelu, alpha=0.1)
                        if dst is None:
                            dd = bass.AP(out_p[:].tensor, b0 * W,
                                         [[RR * W, 64], [W, BAND], [1, W]])
                        else:
                            dd = bass.AP(dst[:].tensor, (b0 + 2) * CW + 2,
                                         [[CWH, mout], [CW, BAND], [1, W]])
                        sv = bass.AP(otile[:].tensor, 2,
                                     [[BAND * CW, mout], [CW, BAND], [1, W]])
                        nc.sync.dma_start(dd, sv)

            def dcn_stage(cvA, cvB):
                BAND = 2
                N = BAND * CW
                q2flat = cv_q2[:].rearrange("c h w -> c (h w)")
                with (tc.tile_pool(name="dsx", bufs=2) as sx,
                      tc.tile_pool(name="dsm", bufs=2) as sm,
                      tc.tile_pool(name="dsa", bufs=2) as sa,
                      tc.tile_pool(name="dso", bufs=2) as so,
                      tc.tile_pool(name="dpd", bufs=2, space="PSUM") as pd,
                      tc.tile_pool(name="dpo", bufs=1, space="PSUM") as po):
                    for b0 in range(0, RR, BAND):
                        xrows = BAND + 4
                        xbase = b0 * CW
                        xpitch = GUARD + xrows * CW + SLACK
                        xts = {}
                        for nm, cv, delta in (("f1", cvA, 1), ("f2", cvA, CW),
                                              ("r1", cvB, 1), ("r2", cvB, CW)):
                            sf = cv[:].rearrange("c h w -> c (h w)")
                            t = sx.tile([128, xpitch], BF16, tag=f"dx{nm}")
                            nc.sync.dma_start(t[0:64, GUARD:GUARD + xrows * CW],
                                              sf[:, xbase:xbase + xrows * CW])
                            nc.sync.dma_start(t[64:128, GUARD:GUARD + xrows * CW],
                                              sf[:, xbase + delta:xbase + delta + xrows * CW])
                            xts[nm] = t
                        orows = BAND + 2
                        obase = (b0 + 1) * CW
                        opitch = GUARD + orows * CW + SLACK
                        omt = {}
                        for nm, half, delta in (("f1", 0, 1), ("f2", 0, CW),
                                                ("r1", 1, 1), ("r2", 1, CW)):
                            t = sx.tile([128, opitch], BF16, tag=f"do{nm}")
                            c0 = 64 * half
                            nc.sync.dma_start(t[0:64, GUARD:GUARD + orows * CW],
                                              q2flat[c0:c0 + 64, obase:obase + orows * CW])
                            nc.sync.dma_start(t[64:128, GUARD:GUARD + orows * CW],
                                              q2flat[c0:c0 + 64, obase + delta:obase + delta + orows * CW])
                            omt[nm] = t

                        alpha9 = {}
                        for px in ("f", "r"):
                            oyt = sm.tile([72, BAND, CW], BF16, tag="oy")
                            oxt = sm.tile([72, BAND, CW], BF16, tag="ox")
                            mt72 = sm.tile([72, BAND, CW], BF16, tag="mt72")
                            for r in range(BAND):
                                accA = po.tile([72, CW], F32, tag="omA")
                                accB = po.tile([72, CW], F32, tag="omB")
                                accC = po.tile([72, CW], F32, tag="omC")
                                for acc, wnm, bnm, mw in ((accA, "womA", "bomA", 72),
                                                          (accB, "womB", "bomB", 72),
                                                          (accC, "womC", "bomC", 72)):
                                    wtile = wt[wnm]
                                    first = True
                                    for s, ky in enumerate((-1, 0, 1)):
                                        off = GUARD + (r + 1 + ky) * CW - 1
                                        rhs = bass.AP(omt[px + "1"][:].tensor, off,
                                                      [[opitch, 128], [1, CW]])
                                        nc.tensor.matmul(acc[:], wtile[:, s, 0:mw], rhs,
                                                         start=first, stop=False)
                                        first = False
                                    off = GUARD + r * CW + 1
                                    rhs = bass.AP(omt[px + "2"][:].tensor, off,
                                                  [[opitch, 128], [1, CW]])
                                    nc.tensor.matmul(acc[:], wtile[:, 3, 0:mw], rhs,
                                                     start=False, stop=False)
                                    off = GUARD + (r + 2) * CW + 1
                                    rhs = bass.AP(omt[px + "1"][:].tensor, off,
                                                  [[opitch, 128], [1, CW]])
                                    nc.tensor.matmul(acc[:], wtile[:, 4, 0:mw], rhs,
                                                     start=False, stop=False)
                                    nc.tensor.matmul(acc[:], wt[bnm][:, 0:mw], ones[:],
                                                     start=False, stop=True)
                                E = 0.999
                                nc.vector.tensor_scalar(oyt[:, r, :], accA[0:72, :],
                                                        E, -E, ALU.min, ALU.max)
                                nc.vector.tensor_scalar(oxt[:, r, :], accB[0:72, :],
                                                        E, -E, ALU.min, ALU.max)
                                nc.scalar.activation(mt72[:, r, :], accC[0:72, :], AF.Sigmoid)
                            oym = sm.tile([72, BAND, CW], BF16, tag="oym")
                            nc.vector.tensor_tensor(oym[:], oyt[:], mt72[:], ALU.mult)
                            wy = sm.tile([72, 3, BAND, CW], BF16, tag="wy")
                            nc.scalar.activation(wy[:, 0, :, :], oym[:], AF.Relu, scale=-1.0)
                            nc.scalar.activation(wy[:, 2, :, :], oym[:], AF.Relu)
                            awy = sm.tile([72, BAND, CW], BF16, tag="awy")
                            nc.scalar.activation(awy[:], oym[:], AF.Abs)
                            nc.vector.tensor_tensor(wy[:, 1, :, :], mt72[:], awy[:], ALU.subtract)
                            wx = sm.tile([72, 3, BAND, CW], BF16, tag="wx")
                            nc.scalar.activation(wx[:, 0, :, :], oxt[:], AF.Relu, scale=-1.0)
                            nc.scalar.activation(wx[:, 2, :, :], oxt[:], AF.Relu)
                            awx = sm.tile([72, BAND, CW], BF16, tag="awx")
                            nc.scalar.activation(awx[:], oxt[:], AF.Abs)
                            nc.vector.tensor_scalar(wx[:, 1, :, :], awx[:], -1.0, 1.0,
                                                    ALU.mult, ALU.add)
                            a9 = sa.tile([72, 9, N], BF16, tag=f"a9{px}")
                            for dy in range(3):
                                for dx in range(3):
                                    nc.vector.tensor_tensor(
                                        a9[:, dy * 3 + dx, :],
                                        wy[:, dy, :, :].rearrange("p a b -> p (a b)"),
                                        wx[:, dx, :, :].rearrange("p a b -> p (a b)"),
                                        ALU.mult)
                            alpha9[px] = a9

                        ddacc = []
                        for r in range(BAND):
                            dt_ = pd.tile([128, CW], F32, tag=f"dd{r}", name=f"ddacc{r}")
                            ddacc.append(dt_)
                        first_mm = [True] * BAND

                        slots = []
                        for px in ("f", "r"):
                            for ky in (-1, 0, 1):
                                k0 = (ky + 1) * 3 + 0
                                k1 = (ky + 1) * 3 + 1
                                slots.append((px, px + "1", ky, -1, k0, k1))
                            slots.append((px, px + "2", -1, 1, 2, 5))

                        for sidx, (px, xnm, bky, bkx, k0, k1) in enumerate(slots):
                            a9 = alpha9[px]
                            widx = sidx if px == "f" else sidx  # slot order matches wd packing
                            arep = sa.tile([128, 9, N], BF16, tag="arep")
                            for hh, kk in ((0, k0), (1, k1)):
                                for cc in range(8):
                                    nc.sync.dma_start(
                                        arep[64 * hh + cc:64 * hh + cc + 57:8, :, :],
                                        a9[kk * 8:kk * 8 + 8, :, :])
                            prod = sa.tile([128, 9, N], BF16, tag="prod")
                            xt = xts[xnm]
                            for dy in range(3):
                                for dx in range(3):
                                    cell = dy * 3 + dx
                                    off = GUARD + (1 + bky + dy) * CW + (bkx + dx - 1)
                                    xv = bass.AP(xt[:].tensor, off, [[xpitch, 128], [1, N]])
                                    nc.vector.tensor_tensor(prod[:, cell, :], xv,
                                                            arep[:, cell, :], ALU.mult)
                            for cell in range(9):
                                for r in range(BAND):
                                    nc.tensor.matmul(ddacc[r][:], wt["wd"][:, widx, :],
                                                     prod[:, cell, r * CW:(r + 1) * CW],
                                                     start=first_mm[r], stop=False)
                                    first_mm[r] = False

                        # merged single slot: fea tap (1,1) k=8 half0, ref half1
                        arep = sa.tile([128, 9, N], BF16, tag="arep")
                        for hh, px in ((0, "f"), (1, "r")):
                            a9 = alpha9[px]
                            for cc in range(8):
                                nc.sync.dma_start(
                                    arep[64 * hh + cc:64 * hh + cc + 57:8, :, :],
                                    a9[64:72, :, :])
                        prod = sa.tile([128, 9, N], BF16, tag="prod")
                        for hh, xnm in ((0, "f1"), (1, "r1")):
                            xt = xts[xnm]
                            for dy in range(3):
                                for dx in range(3):
                                    cell = dy * 3 + dx
                                    off = GUARD + (1 + 1 + dy) * CW + (1 + dx - 1) - hh
                                    xv = bass.AP(xt[:].tensor, off + 64 * hh * xpitch,
                                                 [[xpitch, 64], [1, N]])
                                    ov = bass.AP(prod[:].tensor, 64 * hh * 9 * N + cell * N,
                                                 [[9 * N, 64], [1, N]])
                                    av = bass.AP(arep[:].tensor, 64 * hh * 9 * N + cell * N,
                                                 [[9 * N, 64], [1, N]])
                                    nc.vector.tensor_tensor(ov, xv, av, ALU.mult)
                        for cell in range(9):
                            for r in range(BAND):
                                nc.tensor.matmul(ddacc[r][:], wt["wd"][:, 8, :],
                                                 prod[:, cell, r * CW:(r + 1) * CW],
                                                 start=first_mm[r], stop=False)
                                first_mm[r] = False

                        dout = so.tile([128, BAND, CW], BF16, tag="ddout")
                        for r in range(BAND):
                            nc.tensor.matmul(ddacc[r][:], wt["bd"][:, :], ones[:],
                                             start=False, stop=True)
                            nc.scalar.activation(dout[:, r, :], ddacc[r][:], AF.Prelu, alpha=0.1)
                        dd = bass.AP(cv_dd[:].tensor, (b0 + 2) * CW + 2,
                                     [[CWH, 128], [CW, BAND], [1, W]])
                        sv = bass.AP(dout[:].tensor, 2, [[BAND * CW, 128], [CW, BAND], [1, W]])
                        nc.sync.dma_start(dd, sv)

            def align_block(cvA, cvB, cvO, last=False):
                conv_stage([cvA, cvB], cv_q1, "w1", "b1", 128)
                conv_stage([cv_q1], cv_q2, "w2", "b2", 128)
                dcn_stage(cvA, cvB)
                conv_stage([cv_dd], cv_g, "wf1", "bf1", 64)
                pair_conv_stage(cv_g, None if last else cvO, "wf2", "bf2", 64)

            align_block(cv_in[0], cv_in[1], cv_b1)
            align_block(cv_b1, cv_in[2], cv_b2)
            align_block(cv_in[4], cv_in[3], cv_b3)
            align_block(cv_b2, cv_b3, None, last=True)

    nc.compile()
    return nc


def _pack_weights(p):
    out = {}
    w1 = np.zeros((128, 9, 128), np.float32)
    for tap in range(9):
        ky, kx = tap // 3, tap % 3
        w1[:, tap, 0:64] = p["w_of1"][:, :, ky, kx].T
        w1[0:64, tap, 64:128] = p["w_or1"][:, 64:128, ky, kx].T
        w1[64:128, tap, 64:128] = p["w_or1"][:, 0:64, ky, kx].T
    out["w1"] = w1
    out["b1"] = np.concatenate([p["b_of1"], p["b_or1"]])[None, :]

    w2 = np.zeros((128, 9, 128), np.float32)
    for tap in range(9):
        ky, kx = tap // 3, tap % 3
        w2[0:64, tap, 0:64] = p["w_of2"][:, :, ky, kx].T
        w2[64:128, tap, 64:128] = p["w_or2"][:, :, ky, kx].T
    out["w2"] = w2
    out["b2"] = np.concatenate([p["b_of2"], p["b_or2"]])[None, :]

    w_om, b_om = p["w_om"], p["b_om"]
    oy_ch = np.array([g * 18 + 2 * k for k in range(KK) for g in range(DG)])
    ox_ch = oy_ch + 1
    m_ch = np.array([144 + g * 9 + k for k in range(KK) for g in range(DG)])
    chA, chB, chC = oy_ch, ox_ch, m_ch
    slot_taps = [((0, 0), (0, 1)), ((1, 0), (1, 1)), ((2, 0), (2, 1)),
                 ((0, 2), (1, 2)), ((2, 2), None)]
    for nm, chs, mw in (("womA", chA, 72), ("womB", chB, 72), ("womC", chC, 72)):
        wm = np.zeros((128, 5, mw), np.float32)
        for s, (t0, t1) in enumerate(slot_taps):
            wm[0:64, s, :] = w_om[chs][:, :, t0[0], t0[1]].T
            if t1 is not None:
                wm[64:128, s, :] = w_om[chs][:, :, t1[0], t1[1]].T
        out[nm] = wm
    out["bomA"] = b_om[chA][None, :]
    out["bomB"] = b_om[chB][None, :]
    out["bomC"] = b_om[chC][None, :]

    Wd = p["w_dcn"].reshape(NF, DG, NF // DG, KK)
    wd = np.zeros((128, 9, 128), np.float32)
    pair_ks = [(0, 1), (3, 4), (6, 7), (2, 5)]
    for i, (k0, k1) in enumerate(pair_ks):
        for hh, kk in ((0, k0), (1, k1)):
            blk = Wd[:, :, :, kk].reshape(NF, 64).T
            wd[64 * hh:64 * hh + 64, i, 0:64] = blk
            wd[64 * hh:64 * hh + 64, 4 + i, 64:128] = blk
    blk8 = Wd[:, :, :, 8].reshape(NF, 64).T
    wd[0:64, 8, 0:64] = blk8
    wd[64:128, 8, 64:128] = blk8
    out["wd"] = wd
    out["bd"] = np.concatenate([p["b_dcn"], p["b_dcn"]])[None, :]

    wf1 = np.zeros((128, 9, 64), np.float32)
    for tap in range(9):
        ky, kx = tap // 3, tap % 3
        wf1[:, tap, :] = p["w_f1"][:, :, ky, kx].T
    out["wf1"] = wf1
    out["bf1"] = p["b_f1"][None, :]

    wf2 = np.zeros((128, 5, 64), np.float32)
    for s, (t0, t1) in enumerate(slot_taps):
        wf2[0:64, s, :] = p["w_f2"][:, :, t0[0], t0[1]].T
        if t1 is not None:
            wf2[64:128, s, :] = p["w_f2"][:, :, t1[0], t1[1]].T
    out["wf2"] = wf2
    out["bf2"] = p["b_f2"][None, :]
    return out


class _Runner:
    """Cached PJRT shard_map executor for the Bass program (axon path).

    Mirrors concourse.bass2jax.run_bass_via_pjrt but keeps the jitted
    callable (and the donated output buffer) alive across calls, so only
    input upload + execute + output fetch happen per call.
    """

    def __init__(self, nc, n_cores=8):
        import jax
        import concourse.mybir as mybir
        from jax.sharding import Mesh, PartitionSpec, NamedSharding
        from jax.experimental.shard_map import shard_map
        from concourse.bass2jax import (_bass_exec_p, install_neuronx_cc_hook,
                                        partition_id_tensor)

        install_neuronx_cc_hook()
        self.jax = jax
        self.nc = nc
        self.n_cores = n_cores
        partition_name = nc.partition_id_tensor.name if nc.partition_id_tensor else None
        in_names, out_names, out_avals = [], [], []
        for alloc in nc.m.functions[0].allocations:
            if not isinstance(alloc, mybir.MemoryLocationSet):
                continue
            name = alloc.memorylocations[0].name
            if alloc.kind == "ExternalInput":
                if name != partition_name:
                    in_names.append(name)
            elif alloc.kind == "ExternalOutput":
                out_names.append(name)
                out_avals.append(jax.core.ShapedArray(
                    tuple(alloc.tensor_shape), mybir.dt.np(alloc.dtype)))
        self.in_names, self.out_names, self.out_avals = in_names, out_names, out_avals
        n_params, n_outs = len(in_names), len(out_names)
        all_in = list(in_names) + list(out_names)
        if partition_name is not None:
            all_in.append(partition_name)

        def _body(*args):
            operands = list(args)
            if partition_name is not None:
                operands.append(partition_id_tensor())
            outs = _bass_exec_p.bind(
                *operands,
                out_avals=tuple(out_avals),
                in_names=tuple(all_in),
                out_names=tuple(out_names),
                lowering_input_output_aliases=(),
                sim_require_finite=True,
                sim_require_nnan=True,
                nc=nc,
            )
            return tuple(outs)

        devices = jax.devices()[:n_cores]
        self.mesh = Mesh(np.asarray(devices), ("core",))
        self.shard = NamedSharding(self.mesh, PartitionSpec("core"))
        in_specs = (PartitionSpec("core"),) * (n_params + n_outs)
        out_specs = (PartitionSpec("core"),) * n_outs
        self.fn = jax.jit(
            shard_map(_body, mesh=self.mesh, in_specs=in_specs,
                      out_specs=out_specs, check_rep=False),
            donate_argnums=tuple(range(n_params, n_params + n_outs)),
            keep_unused=True,
        )
        self.dev_outs = None

    def __call__(self, global_ins: dict):
        if self.dev_outs is None:
            self.dev_outs = [
                self.jax.device_put(
                    np.zeros((self.n_cores * a.shape[0], *a.shape[1:]), a.dtype),
                    self.shard)
                for a in self.out_avals]
        args = [global_ins[n] for n in self.in_names] + list(self.dev_outs)
        outs = self.fn(*args)
        self.dev_outs = list(outs)
        return {n: outs[i] for i, n in enumerate(self.out_names)}


def _quant_frame(x):
    """Per-(batch,channel) symmetric int8 quantization of one frame,
    laid out as the global per-core shard array [512, RR, W]."""
    s = np.maximum(np.maximum(x.max(axis=(2, 3)), -x.min(axis=(2, 3))),
                   1e-20)                                    # [B, 64]
    # |x|<=s so |x*127/s| <= 127 (+epsilon): rint lands in int8 range
    t = x * (127.0 / s)[:, :, None, None]
    np.rint(t, out=t)
    q = t.astype(np.int8)
    g = np.empty((8 * 64, RR, W), np.int8)
    for core in range(8):
        b, hh = core // 2, core % 2
        r0 = 0 if hh == 0 else H - RR
        g[core * 64:(core + 1) * 64] = q[b, :, r0:r0 + RR, :]
    return g, s


def kernel(**inputs):
    import jax
    from concurrent.futures import ThreadPoolExecutor

    if "runner" not in _cache:
        _cache["runner"] = _Runner(_build())
        # 3 workers: frame 0's quant finishes (and its upload starts) sooner,
        # so transfers overlap the remaining quants
        _cache["pool"] = ThreadPoolExecutor(3)
    runner = _cache["runner"]
    pool = _cache["pool"]

    p = {k: np.asarray(v, dtype=np.float32) for k, v in inputs.items()}
    futs = [pool.submit(_quant_frame, p[f"fea{i}"]) for i in range(5)]

    import hashlib
    hsh = hashlib.blake2b(digest_size=16)
    for k in sorted(p):
        if not k.startswith("fea"):
            hsh.update(p[k].tobytes())
    bh = hsh.digest()
    if _cache.get("wblob_hash") != bh:
        wpk = _pack_weights(p)
        blob = np.concatenate([wpk[n].ravel() for n, _ in WSPEC]).astype(BF)
        wblob_g = np.tile(blob, 8)
        _cache["wblob_dev"] = jax.device_put(wblob_g, runner.shard)  # async
        _cache["wblob_hash"] = bh
    gi = {"wblob": _cache["wblob_dev"]}
    fs_g = np.zeros((8 * 64, 8), np.float32)
    for i in range(5):
        g, s = futs[i].result()
        gi[f"feaq{i}"] = jax.device_put(g, runner.shard)     # overlaps later quants
        for b in range(B):
            for hh in range(2):
                core = 2 * b + hh
                fs_g[core * 64:(core + 1) * 64, i] = s[b] / 127.0
    gi["fscale"] = fs_g

    outs = runner(gi)
    res = np.asarray(outs["out"])                           # [512, RR, W] bf16
    out = np.empty((B, NF, H, W), np.float32)
    for core in range(8):
        b, hh = core // 2, core % 2
        blk = res[core * 64:(core + 1) * 64]
        if hh == 0:
            out[b, :, 0:96, :] = blk[:, 0:96, :]
        else:
            out[b, :, 96:192, :] = blk[:, RR - 96:RR, :]
    return out


# revision 10
# speedup vs baseline: 1.3967x; 1.3967x over previous
"""AlignNet (dense CNN + DCNv2) Trainium2 Bass kernel, 8 NeuronCores.

Sharding: data-parallel over (batch, H-half): core c=(b,h) computes output
rows [0:96)/[96:192) of batch b with a 16-row replicated halo (no
inter-core communication).

Transfer-optimized I/O (the axon tunnel is the bottleneck):
  - frame activations shipped as per-(batch,channel)-scaled int8, dequantized
    on device by ActE with a per-partition AP scale -> bf16 canvases
  - all weights packed into one bf16 blob, unpacked by strided DMA views
  - output returned as bf16; donated output buffer lives on device between
    calls so no zero-upload is needed
  - the jitted shard_map executable is cached across kernel() calls

Per-core pipeline (bf16 compute, fp32 PSUM):
  - activations in padded DRAM canvases [C, 118, 324] bf16 (image origin
    (2,2); borders zero = conv/sampling zero-pad)
  - 3x3 convs: 9 (or 5 tap-paired) accumulated matmuls on shifted flat views
  - DCNv2: offsets clipped to (-1,1) -> exact 3x3 hat window; per-(g,k)
    window weights on 72 partitions, replicated to channel layout by
    SBUF->SBUF DMAs, DVE products, 9-cell reduction + channel einsum
    absorbed into TensorE matmuls.
"""
import numpy as np
import ml_dtypes

NF, DG, KK = 64, 8, 9
B, H, W = 4, 192, 320
RR = 112                  # compute rows per core (96 + 16 halo)
CH, CW = RR + 6, W + 4    # canvas 118 x 324, image origin (2,2)
CWH = CH * CW
GUARD = 8
SLACK = 336
BF = ml_dtypes.bfloat16

# weight blob layout: (name, shape) in fixed order
WSPEC = [
    ("w1", (128, 9, 128)), ("b1", (1, 128)),
    ("w2", (128, 9, 128)), ("b2", (1, 128)),
    ("womA", (128, 5, 72)), ("womB", (128, 5, 72)), ("womC", (128, 5, 72)),
    ("bomA", (1, 72)), ("bomB", (1, 72)), ("bomC", (1, 72)),
    ("wd", (128, 9, 128)), ("bd", (1, 128)),
    ("wf1", (128, 9, 64)), ("bf1", (1, 64)),
    ("wf2", (128, 5, 64)), ("bf2", (1, 64)),
]
WOFF = {}
_o = 0
for _n, _s in WSPEC:
    WOFF[_n] = _o
    _o += int(np.prod(_s))
NW = _o

_cache = {}


def _build():
    import concourse.bass as bass
    import concourse.bacc as bacc
    import concourse.mybir as mybir
    from concourse import tile

    F32 = mybir.dt.float32
    BF16 = mybir.dt.bfloat16
    I8 = mybir.dt.int8
    AF = mybir.ActivationFunctionType
    ALU = mybir.AluOpType

    nc = bacc.Bacc("TRN2", target_bir_lowering=False, debug=False)

    feaq = [nc.declare_dram_parameter(f"feaq{i}", [64, RR, W], I8, isOutput=False)
            for i in range(5)]
    fscale = nc.declare_dram_parameter("fscale", [64, 8], F32, isOutput=False)
    wblob = nc.declare_dram_parameter("wblob", [NW], BF16, isOutput=False)
    out_p = nc.declare_dram_parameter("out", [64, RR, W], BF16, isOutput=True)

    def canvas(name, ch):
        return nc.dram_tensor(name, [ch, CH, CW], BF16)

    cv_in = [canvas(f"cv_fea{i}", 64) for i in range(5)]
    cv_b1 = canvas("cv_b1", 64)
    cv_b2 = canvas("cv_b2", 64)
    cv_b3 = canvas("cv_b3", 64)
    cv_q1 = canvas("cv_q1", 128)
    cv_q2 = canvas("cv_q2", 128)
    cv_dd = canvas("cv_dd", 128)
    cv_g = canvas("cv_g", 64)

    with tile.TileContext(nc) as tc:
        with tc.tile_pool(name="wgt", bufs=1) as wgt:
            # ---- unpack bf16 weights from the blob ----
            wt = {}
            for name, shp in WSPEC:
                p_, a_ = shp[0], shp[1]
                b_ = shp[2] if len(shp) == 3 else None
                t16 = wgt.tile(list(shp), BF16, tag=f'w_{name}', name=f'w_{name}')
                if b_ is None:
                    src = bass.AP(wblob[:].tensor, WOFF[name], [[a_, p_], [1, a_]])
                else:
                    src = bass.AP(wblob[:].tensor, WOFF[name],
                                  [[a_ * b_, p_], [b_, a_], [1, b_]])
                nc.sync.dma_start(t16[:], src)
                wt[name] = t16
            fst = wgt.tile([64, 8], F32, tag="fst")
            nc.sync.dma_start(fst[:], fscale[:])
            ones = wgt.tile([1, CW], BF16)
            nc.gpsimd.memset(ones[:], 1.0)

            # ---- zero canvases + dequantize inputs into canvases ----
            with tc.tile_pool(name="init", bufs=2) as ip:
                zt = ip.tile([128, 8192], BF16, tag="zt")
                nc.gpsimd.memset(zt[:], 0.0)
                for cv, ch in ([(c, 64) for c in cv_in] +
                               [(cv_b1, 64), (cv_b2, 64), (cv_b3, 64), (cv_g, 64),
                                (cv_q1, 128), (cv_q2, 128), (cv_dd, 128)]):
                    flat = cv[:].rearrange("c h w -> c (h w)")
                    for o in range(0, CWH, 8192):
                        n = min(8192, CWH - o)
                        nc.sync.dma_start(flat[0:ch, o:o + n], zt[0:ch, 0:n])
                for i in range(5):
                    for r0 in range(0, RR, 16):
                        ti8 = ip.tile([64, 16 * W], I8, tag="qi")
                        src = bass.AP(feaq[i][:].tensor, r0 * W,
                                      [[RR * W, 64], [1, 16 * W]])
                        nc.sync.dma_start(ti8[:], src)
                        t16 = ip.tile([64, 16 * W], BF16, tag="qc")
                        nc.scalar.mul(t16[:], ti8[:], fst[:, i:i + 1])
                        dst = bass.AP(cv_in[i][:].tensor, (r0 + 2) * CW + 2,
                                      [[CWH, 64], [CW, 16], [1, W]])
                        nc.sync.dma_start(dst, t16[:].rearrange("c (r w) -> c r w", r=16))

            # ============ stage helpers ============
            def conv_stage(src_list, dst, w_name, b_name, mout):
                BAND = 8
                wtile = wt[w_name]
                btile = wt[b_name]
                with (tc.tile_pool(name="cs", bufs=2) as sp,
                      tc.tile_pool(name="cps", bufs=3, space="PSUM") as pp):
                    for b0 in range(0, RR, BAND):
                        rows = BAND + 2
                        pitch = GUARD + rows * CW + SLACK
                        xt = sp.tile([128, pitch], BF16, tag="cx")
                        base = (b0 + 1) * CW
                        if len(src_list) == 1:
                            sf = src_list[0][:].rearrange("c h w -> c (h w)")
                            nc.sync.dma_start(xt[:, GUARD:GUARD + rows * CW],
                                              sf[:, base:base + rows * CW])
                        else:
                            for hh in (0, 1):
                                sf = src_list[hh][:].rearrange("c h w -> c (h w)")
                                nc.sync.dma_start(xt[64 * hh:64 * hh + 64, GUARD:GUARD + rows * CW],
                                                  sf[:, base:base + rows * CW])
                        otile = sp.tile([mout, BAND, CW], BF16, tag="co")
                        for r in range(BAND):
                            acc = pp.tile([mout, CW], F32, tag="cp")
                            for tap in range(9):
                                ky, kx = tap // 3 - 1, tap % 3 - 1
                                off = GUARD + (r + 1 + ky) * CW + kx
                                rhs = bass.AP(xt[:].tensor, off, [[pitch, 128], [1, CW]])
                                nc.tensor.matmul(acc[:], wtile[:, tap, 0:mout], rhs,
                                                 start=(tap == 0), stop=False)
                            nc.tensor.matmul(acc[:], btile[:, 0:mout], ones[:],
                                             start=False, stop=True)
                            nc.scalar.activation(otile[:, r, :], acc[:], AF.Prelu, alpha=0.1)
                        if dst is None:
                            dd = bass.AP(out_p[:].tensor, b0 * W,
                                         [[RR * W, 64], [W, BAND], [1, W]])
                        else:
                            dd = bass.AP(dst[:].tensor, (b0 + 2) * CW + 2,
                                         [[CWH, mout], [CW, BAND], [1, W]])
                        sv = bass.AP(otile[:].tensor, 2,
                                     [[BAND * CW, mout], [CW, BAND], [1, W]])
                        nc.sync.dma_start(dd, sv)

            def pair_conv_stage(src, dst, w_name, b_name, mout):
                BAND = 8
                wtile = wt[w_name]
                btile = wt[b_name]
                sflat = src[:].rearrange("c h w -> c (h w)")
                with (tc.tile_pool(name="pcs", bufs=2) as sp,
                      tc.tile_pool(name="pps", bufs=3, space="PSUM") as pp):
                    for b0 in range(0, RR, BAND):
                        rows = BAND + 2
                        base = (b0 + 1) * CW
                        pitch = GUARD + rows * CW + SLACK
                        t1 = sp.tile([128, pitch], BF16, tag="p1")
                        nc.sync.dma_start(t1[0:64, GUARD:GUARD + rows * CW],
                                          sflat[:, base:base + rows * CW])
                        nc.sync.dma_start(t1[64:128, GUARD:GUARD + rows * CW],
                                          sflat[:, base + 1:base + 1 + rows * CW])
                        t2 = sp.tile([128, pitch], BF16, tag="p2")
                        nc.sync.dma_start(t2[0:64, GUARD:GUARD + rows * CW],
                                          sflat[:, base:base + rows * CW])
                        nc.sync.dma_start(t2[64:128, GUARD:GUARD + rows * CW],
                                          sflat[:, base + CW:base + CW + rows * CW])
                        otile = sp.tile([mout, BAND, CW], BF16, tag="po")
                        for r in range(BAND):
                            acc = pp.tile([mout, CW], F32, tag="pp")
                            first = True
                            for s, ky in enumerate((-1, 0, 1)):
                                off = GUARD + (r + 1 + ky) * CW - 1
                                rhs = bass.AP(t1[:].tensor, off, [[pitch, 128], [1, CW]])
                                nc.tensor.matmul(acc[:], wtile[:, s, 0:mout], rhs,
                                                 start=first, stop=False)
                                first = False
                            off = GUARD + r * CW + 1
                            rhs = bass.AP(t2[:].tensor, off, [[pitch, 128], [1, CW]])
                            nc.tensor.matmul(acc[:], wtile[:, 3, 0:mout], rhs, start=False, stop=False)
                            off = GUARD + (r + 2) * CW + 1
                            rhs = bass.AP(t1[:].tensor, off, [[pitch, 128], [1, CW]])
                            nc.tensor.matmul(acc[:], wtile[:, 4, 0:mout], rhs, start=False, stop=False)
                            nc.tensor.matmul(acc[:], btile[:, 0:mout], ones[:], start=False, stop=True)
                            nc.scalar.activation(otile[:, r, :], acc[:], AF.Prelu, alpha=0.1)
                        if dst is None:
                            dd = bass.AP(out_p[:].tensor, b0 * W,
                                         [[RR * W, 64], [W, BAND], [1, W]])
                        else:
                            dd = bass.AP(dst[:].tensor, (b0 + 2) * CW + 2,
                                         [[CWH, mout], [CW, BAND], [1, W]])
                        sv = bass.AP(otile[:].tensor, 2,
                                     [[BAND * CW, mout], [CW, BAND], [1, W]])
                        nc.sync.dma_start(dd, sv)

            def dcn_stage(cvA, cvB):
                BAND = 2
                N = BAND * CW
                q2flat = cv_q2[:].rearrange("c h w -> c (h w)")
                with (tc.tile_pool(name="dsx", bufs=2) as sx,
                      tc.tile_pool(name="dsm", bufs=2) as sm,
                      tc.tile_pool(name="dsa", bufs=2) as sa,
                      tc.tile_pool(name="dso", bufs=2) as so,
                      tc.tile_pool(name="dpd", bufs=2, space="PSUM") as pd,
                      tc.tile_pool(name="dpo", bufs=1, space="PSUM") as po):
                    for b0 in range(0, RR, BAND):
                        xrows = BAND + 4
                        xbase = b0 * CW
                        xpitch = GUARD + xrows * CW + SLACK
                        xts = {}
                        for nm, cv, delta in (("f1", cvA, 1), ("f2", cvA, CW),
                                              ("r1", cvB, 1), ("r2", cvB, CW)):
                            sf = cv[:].rearrange("c h w -> c (h w)")
                            t = sx.tile([128, xpitch], BF16, tag=f"dx{nm}")
                            nc.sync.dma_start(t[0:64, GUARD:GUARD + xrows * CW],
                                              sf[:, xbase:xbase + xrows * CW])
                            nc.sync.dma_start(t[64:128, GUARD:GUARD + xrows * CW],
                                              sf[:, xbase + delta:xbase + delta + xrows * CW])
                            xts[nm] = t
                        orows = BAND + 2
                        obase = (b0 + 1) * CW
                        opitch = GUARD + orows * CW + SLACK
                        omt = {}
                        for nm, half, delta in (("f1", 0, 1), ("f2", 0, CW),
                                                ("r1", 1, 1), ("r2", 1, CW)):
                            t = sx.tile([128, opitch], BF16, tag=f"do{nm}")
                            c0 = 64 * half
                            nc.sync.dma_start(t[0:64, GUARD:GUARD + orows * CW],
                                              q2flat[c0:c0 + 64, obase:obase + orows * CW])
                            nc.sync.dma_start(t[64:128, GUARD:GUARD + orows * CW],
                                              q2flat[c0:c0 + 64, obase + delta:obase + delta + orows * CW])
                            omt[nm] = t

                        alpha9 = {}
                        for px in ("f", "r"):
                            oyt = sm.tile([72, BAND, CW], BF16, tag="oy")
                            oxt = sm.tile([72, BAND, CW], BF16, tag="ox")
                            mt72 = sm.tile([72, BAND, CW], BF16, tag="mt72")
                            for r in range(BAND):
                                accA = po.tile([72, CW], F32, tag="omA")
                                accB = po.tile([72, CW], F32, tag="omB")
                                accC = po.tile([72, CW], F32, tag="omC")
                                for acc, wnm, bnm, mw in ((accA, "womA", "bomA", 72),
                                                          (accB, "womB", "bomB", 72),
                                                          (accC, "womC", "bomC", 72)):
                                    wtile = wt[wnm]
                                    first = True
                                    for s, ky in enumerate((-1, 0, 1)):
                                        off = GUARD + (r + 1 + ky) * CW - 1
                                        rhs = bass.AP(omt[px + "1"][:].tensor, off,
                                                      [[opitch, 128], [1, CW]])
                                        nc.tensor.matmul(acc[:], wtile[:, s, 0:mw], rhs,
                                                         start=first, stop=False)
                                        first = False
                                    off = GUARD + r * CW + 1
                                    rhs = bass.AP(omt[px + "2"][:].tensor, off,
                                                  [[opitch, 128], [1, CW]])
                                    nc.tensor.matmul(acc[:], wtile[:, 3, 0:mw], rhs,
                                                     start=False, stop=False)
                                    off = GUARD + (r + 2) * CW + 1
                                    rhs = bass.AP(omt[px + "1"][:].tensor, off,
                                                  [[opitch, 128], [1, CW]])
                                    nc.tensor.matmul(acc[:], wtile[:, 4, 0:mw], rhs,
                                                     start=False, stop=False)
                                    nc.tensor.matmul(acc[:], wt[bnm][:, 0:mw], ones[:],
                                                     start=False, stop=True)
                                E = 0.999
                                nc.vector.tensor_scalar(oyt[:, r, :], accA[0:72, :],
                                                        E, -E, ALU.min, ALU.max)
                                nc.vector.tensor_scalar(oxt[:, r, :], accB[0:72, :],
                                                        E, -E, ALU.min, ALU.max)
                                nc.scalar.activation(mt72[:, r, :], accC[0:72, :], AF.Sigmoid)
                            oym = sm.tile([72, BAND, CW], BF16, tag="oym")
                            nc.vector.tensor_tensor(oym[:], oyt[:], mt72[:], ALU.mult)
                            wy = sm.tile([72, 3, BAND, CW], BF16, tag="wy")
                            nc.scalar.activation(wy[:, 0, :, :], oym[:], AF.Relu, scale=-1.0)
                            nc.scalar.activation(wy[:, 2, :, :], oym[:], AF.Relu)
                            awy = sm.tile([72, BAND, CW], BF16, tag="awy")
                            nc.scalar.activation(awy[:], oym[:], AF.Abs)
                            nc.vector.tensor_tensor(wy[:, 1, :, :], mt72[:], awy[:], ALU.subtract)
                            wx = sm.tile([72, 3, BAND, CW], BF16, tag="wx")
                            nc.scalar.activation(wx[:, 0, :, :], oxt[:], AF.Relu, scale=-1.0)
                            nc.scalar.activation(wx[:, 2, :, :], oxt[:], AF.Relu)
                            awx = sm.tile([72, BAND, CW], BF16, tag="awx")
                            nc.scalar.activation(awx[:], oxt[:], AF.Abs)
                            nc.vector.tensor_scalar(wx[:, 1, :, :], awx[:], -1.0, 1.0,
                                                    ALU.mult, ALU.add)
                            a9 = sa.tile([72, 9, N], BF16, tag=f"a9{px}")
                            for dy in range(3):
                                for dx in range(3):
                                    nc.vector.tensor_tensor(
                                        a9[:, dy * 3 + dx, :],
                                        wy[:, dy, :, :].rearrange("p a b -> p (a b)"),
                                        wx[:, dx, :, :].rearrange("p a b -> p (a b)"),
                                        ALU.mult)
                            alpha9[px] = a9

                        ddacc = []
                        for r in range(BAND):
                            dt_ = pd.tile([128, CW], F32, tag=f"dd{r}", name=f"ddacc{r}")
                            ddacc.append(dt_)
                        first_mm = [True] * BAND

                        slots = []
                        for px in ("f", "r"):
                            for ky in (-1, 0, 1):
                                k0 = (ky + 1) * 3 + 0
                                k1 = (ky + 1) * 3 + 1
                                slots.append((px, px + "1", ky, -1, k0, k1))
                            slots.append((px, px + "2", -1, 1, 2, 5))

                        for sidx, (px, xnm, bky, bkx, k0, k1) in enumerate(slots):
                            a9 = alpha9[px]
                            widx = sidx if px == "f" else sidx  # slot order matches wd packing
                            arep = sa.tile([128, 9, N], BF16, tag="arep")
                            for hh, kk in ((0, k0), (1, k1)):
                                for cc in range(8):
                                    nc.sync.dma_start(
                                        arep[64 * hh + cc:64 * hh + cc + 57:8, :, :],
                                        a9[kk * 8:kk * 8 + 8, :, :])
                            prod = sa.tile([128, 9, N], BF16, tag="prod")
                            xt = xts[xnm]
                            for dy in range(3):
                                for dx in range(3):
                                    cell = dy * 3 + dx
                                    off = GUARD + (1 + bky + dy) * CW + (bkx + dx - 1)
                                    xv = bass.AP(xt[:].tensor, off, [[xpitch, 128], [1, N]])
                                    nc.vector.tensor_tensor(prod[:, cell, :], xv,
                                                            arep[:, cell, :], ALU.mult)
                            for cell in range(9):
                                for r in range(BAND):
                                    nc.tensor.matmul(ddacc[r][:], wt["wd"][:, widx, :],
                                                     prod[:, cell, r * CW:(r + 1) * CW],
                                                     start=first_mm[r], stop=False)
                                    first_mm[r] = False

                        # merged single slot: fea tap (1,1) k=8 half0, ref half1
                        arep = sa.tile([128, 9, N], BF16, tag="arep")
                        for hh, px in ((0, "f"), (1, "r")):
                            a9 = alpha9[px]
                            for cc in range(8):
                                nc.sync.dma_start(
                                    arep[64 * hh + cc:64 * hh + cc + 57:8, :, :],
                                    a9[64:72, :, :])
                        prod = sa.tile([128, 9, N], BF16, tag="prod")
                        for hh, xnm in ((0, "f1"), (1, "r1")):
                            xt = xts[xnm]
                            for dy in range(3):
                                for dx in range(3):
                                    cell = dy * 3 + dx
                                    off = GUARD + (1 + 1 + dy) * CW + (1 + dx - 1) - hh
                                    xv = bass.AP(xt[:].tensor, off + 64 * hh * xpitch,
                                                 [[xpitch, 64], [1, N]])
                                    ov = bass.AP(prod[:].tensor, 64 * hh * 9 * N + cell * N,
                                                 [[9 * N, 64], [1, N]])
                                    av = bass.AP(arep[:].tensor, 64 * hh * 9 * N + cell * N,
                                                 [[9 * N, 64], [1, N]])
                                    nc.vector.tensor_tensor(ov, xv, av, ALU.mult)
                        for cell in range(9):
                            for r in range(BAND):
                                nc.tensor.matmul(ddacc[r][:], wt["wd"][:, 8, :],
                                                 prod[:, cell, r * CW:(r + 1) * CW],
                                                 start=first_mm[r], stop=False)
                                first_mm[r] = False

                        dout = so.tile([128, BAND, CW], BF16, tag="ddout")
                        for r in range(BAND):
                            nc.tensor.matmul(ddacc[r][:], wt["bd"][:, :], ones[:],
                                             start=False, stop=True)
                            nc.scalar.activation(dout[:, r, :], ddacc[r][:], AF.Prelu, alpha=0.1)
                        dd = bass.AP(cv_dd[:].tensor, (b0 + 2) * CW + 2,
                                     [[CWH, 128], [CW, BAND], [1, W]])
                        sv = bass.AP(dout[:].tensor, 2, [[BAND * CW, 128], [CW, BAND], [1, W]])
                        nc.sync.dma_start(dd, sv)

            def align_block(cvA, cvB, cvO, last=False):
                conv_stage([cvA, cvB], cv_q1, "w1", "b1", 128)
                conv_stage([cv_q1], cv_q2, "w2", "b2", 128)
                dcn_stage(cvA, cvB)
                conv_stage([cv_dd], cv_g, "wf1", "bf1", 64)
                pair_conv_stage(cv_g, None if last else cvO, "wf2", "bf2", 64)

            align_block(cv_in[0], cv_in[1], cv_b1)
            align_block(cv_b1, cv_in[2], cv_b2)
            align_block(cv_in[4], cv_in[3], cv_b3)
            align_block(cv_b2, cv_b3, None, last=True)

    nc.compile()
    return nc


def _pack_weights(p):
    out = {}
    w1 = np.zeros((128, 9, 128), np.float32)
    for tap in range(9):
        ky, kx = tap // 3, tap % 3
        w1[:, tap, 0:64] = p["w_of1"][:, :, ky, kx].T
        w1[0:64, tap, 64:128] = p["w_or1"][:, 64:128, ky, kx].T
        w1[64:128, tap, 64:128] = p["w_or1"][:, 0:64, ky, kx].T
    out["w1"] = w1
    out["b1"] = np.concatenate([p["b_of1"], p["b_or1"]])[None, :]

    w2 = np.zeros((128, 9, 128), np.float32)
    for tap in range(9):
        ky, kx = tap // 3, tap % 3
        w2[0:64, tap, 0:64] = p["w_of2"][:, :, ky, kx].T
        w2[64:128, tap, 64:128] = p["w_or2"][:, :, ky, kx].T
    out["w2"] = w2
    out["b2"] = np.concatenate([p["b_of2"], p["b_or2"]])[None, :]

    w_om, b_om = p["w_om"], p["b_om"]
    oy_ch = np.array([g * 18 + 2 * k for k in range(KK) for g in range(DG)])
    ox_ch = oy_ch + 1
    m_ch = np.array([144 + g * 9 + k for k in range(KK) for g in range(DG)])
    chA, chB, chC = oy_ch, ox_ch, m_ch
    slot_taps = [((0, 0), (0, 1)), ((1, 0), (1, 1)), ((2, 0), (2, 1)),
                 ((0, 2), (1, 2)), ((2, 2), None)]
    for nm, chs, mw in (("womA", chA, 72), ("womB", chB, 72), ("womC", chC, 72)):
        wm = np.zeros((128, 5, mw), np.float32)
        for s, (t0, t1) in enumerate(slot_taps):
            wm[0:64, s, :] = w_om[chs][:, :, t0[0], t0[1]].T
            if t1 is not None:
                wm[64:128, s, :] = w_om[chs][:, :, t1[0], t1[1]].T
        out[nm] = wm
    out["bomA"] = b_om[chA][None, :]
    out["bomB"] = b_om[chB][None, :]
    out["bomC"] = b_om[chC][None, :]

    Wd = p["w_dcn"].reshape(NF, DG, NF // DG, KK)
    wd = np.zeros((128, 9, 128), np.float32)
    pair_ks = [(0, 1), (3, 4), (6, 7), (2, 5)]
    for i, (k0, k1) in enumerate(pair_ks):
        for hh, kk in ((0, k0), (1, k1)):
            blk = Wd[:, :, :, kk].reshape(NF, 64).T
            wd[64 * hh:64 * hh + 64, i, 0:64] = blk
            wd[64 * hh:64 * hh + 64, 4 + i, 64:128] = blk
    blk8 = Wd[:, :, :, 8].reshape(NF, 64).T
    wd[0:64, 8, 0:64] = blk8
    wd[64:128, 8, 64:128] = blk8
    out["wd"] = wd
    out["bd"] = np.concatenate([p["b_dcn"], p["b_dcn"]])[None, :]

    wf1 = np.zeros((128, 9, 64), np.float32)
    for tap in range(9):
        ky, kx = tap // 3, tap % 3
        wf1[:, tap, :] = p["w_f1"][:, :, ky, kx].T
    out["wf1"] = wf1
    out["bf1"] = p["b_f1"][None, :]

    wf2 = np.zeros((128, 5, 64), np.float32)
    for s, (t0, t1) in enumerate(slot_taps):
        wf2[0:64, s, :] = p["w_f2"][:, :, t0[0], t0[1]].T
        if t1 is not None:
            wf2[64:128, s, :] = p["w_f2"][:, :, t1[0], t1[1]].T
    out["wf2"] = wf2
    out["bf2"] = p["b_f2"][None, :]
    return out


class _Runner:
    """Cached PJRT shard_map executor for the Bass program (axon path).

    Mirrors concourse.bass2jax.run_bass_via_pjrt but keeps the jitted
    callable (and the donated output buffer) alive across calls, so only
    input upload + execute + output fetch happen per call.
    """

    def __init__(self, nc, n_cores=8):
        import jax
        import concourse.mybir as mybir
        from jax.sharding import Mesh, PartitionSpec, NamedSharding
        from jax.experimental.shard_map import shard_map
        from concourse.bass2jax import (_bass_exec_p, install_neuronx_cc_hook,
                                        partition_id_tensor)

        install_neuronx_cc_hook()
        self.jax = jax
        self.nc = nc
        self.n_cores = n_cores
        partition_name = nc.partition_id_tensor.name if nc.partition_id_tensor else None
        in_names, out_names, out_avals = [], [], []
        for alloc in nc.m.functions[0].allocations:
            if not isinstance(alloc, mybir.MemoryLocationSet):
                continue
            name = alloc.memorylocations[0].name
            if alloc.kind == "ExternalInput":
                if name != partition_name:
                    in_names.append(name)
            elif alloc.kind == "ExternalOutput":
                out_names.append(name)
                out_avals.append(jax.core.ShapedArray(
                    tuple(alloc.tensor_shape), mybir.dt.np(alloc.dtype)))
        self.in_names, self.out_names, self.out_avals = in_names, out_names, out_avals
        n_params, n_outs = len(in_names), len(out_names)
        all_in = list(in_names) + list(out_names)
        if partition_name is not None:
            all_in.append(partition_name)

        def _body(*args):
            operands = list(args)
            if partition_name is not None:
                operands.append(partition_id_tensor())
            outs = _bass_exec_p.bind(
                *operands,
                out_avals=tuple(out_avals),
                in_names=tuple(all_in),
                out_names=tuple(out_names),
                lowering_input_output_aliases=(),
                sim_require_finite=True,
                sim_require_nnan=True,
                nc=nc,
            )
            return tuple(outs)

        devices = jax.devices()[:n_cores]
        self.mesh = Mesh(np.asarray(devices), ("core",))
        self.shard = NamedSharding(self.mesh, PartitionSpec("core"))
        in_specs = (PartitionSpec("core"),) * (n_params + n_outs)
        out_specs = (PartitionSpec("core"),) * n_outs
        self.fn = jax.jit(
            shard_map(_body, mesh=self.mesh, in_specs=in_specs,
                      out_specs=out_specs, check_rep=False),
            donate_argnums=tuple(range(n_params, n_params + n_outs)),
            keep_unused=True,
        )
        self.dev_outs = None

    def __call__(self, global_ins: dict):
        if self.dev_outs is None:
            self.dev_outs = [
                self.jax.device_put(
                    np.zeros((self.n_cores * a.shape[0], *a.shape[1:]), a.dtype),
                    self.shard)
                for a in self.out_avals]
        args = [global_ins[n] for n in self.in_names] + list(self.dev_outs)
        outs = self.fn(*args)
        self.dev_outs = list(outs)
        return {n: outs[i] for i, n in enumerate(self.out_names)}


_tls_buffers = {}


def _quant_frame(x):
    """Per-(batch,channel) symmetric int8 quantization of one frame,
    laid out as the global per-core shard array [512, RR, W]."""
    import threading
    s = np.maximum(np.maximum(x.max(axis=(2, 3)), -x.min(axis=(2, 3))),
                   1e-20)                                    # [B, 64]
    # per-thread scratch: t/q never escape this thread (g is copied out
    # before return), so cross-call reuse avoids 80MB of page faults
    tid = threading.get_ident()
    bufs = _tls_buffers.get(tid)
    if bufs is None or bufs[0].shape != x.shape:
        bufs = (np.empty(x.shape, np.float32), np.empty(x.shape, np.int8))
        _tls_buffers[tid] = bufs
    t, q = bufs
    # |x|<=s so |x*127/s| <= 127 (+epsilon): rint lands in int8 range
    np.multiply(x, (127.0 / s)[:, :, None, None], out=t)
    np.rint(t, out=q, casting="unsafe")
    g = np.empty((8 * 64, RR, W), np.int8)
    for core in range(8):
        b, hh = core // 2, core % 2
        r0 = 0 if hh == 0 else H - RR
        g[core * 64:(core + 1) * 64] = q[b, :, r0:r0 + RR, :]
    return g, s


def kernel(**inputs):
    import jax
    from concurrent.futures import ThreadPoolExecutor

    if "runner" not in _cache:
        _cache["runner"] = _Runner(_build())
        # 3 workers: frame 0's quant finishes (and its upload starts) sooner,
        # so transfers overlap the remaining quants
        _cache["pool"] = ThreadPoolExecutor(3)
    runner = _cache["runner"]
    pool = _cache["pool"]

    p = {k: np.asarray(v, dtype=np.float32) for k, v in inputs.items()}
    futs = [pool.submit(_quant_frame, p[f"fea{i}"]) for i in range(5)]

    import hashlib
    hsh = hashlib.blake2b(digest_size=16)
    for k in sorted(p):
        if not k.startswith("fea"):
            hsh.update(p[k].tobytes())
    bh = hsh.digest()
    if _cache.get("wblob_hash") != bh:
        wpk = _pack_weights(p)
        blob = np.concatenate([wpk[n].ravel() for n, _ in WSPEC]).astype(BF)
        wblob_g = np.tile(blob, 8)
        _cache["wblob_dev"] = jax.device_put(wblob_g, runner.shard)  # async
        _cache["wblob_hash"] = bh
    gi = {"wblob": _cache["wblob_dev"]}
    fs_g = np.zeros((8 * 64, 8), np.float32)
    for i in range(5):
        g, s = futs[i].result()
        gi[f"feaq{i}"] = jax.device_put(g, runner.shard)     # overlaps later quants
        for b in range(B):
            for hh in range(2):
                core = 2 * b + hh
                fs_g[core * 64:(core + 1) * 64, i] = s[b] / 127.0
    gi["fscale"] = fs_g

    outs = runner(gi)
    res = np.asarray(outs["out"])                           # [512, RR, W] bf16
    out = np.empty((B, NF, H, W), np.float32)
    for core in range(8):
        b, hh = core // 2, core % 2
        blk = res[core * 64:(core + 1) * 64]
        if hh == 0:
            out[b, :, 0:96, :] = blk[:, 0:96, :]
        else:
            out[b, :, 96:192, :] = blk[:, RR - 96:RR, :]
    return out
